# revision 32
# baseline (speedup 1.0000x reference)
"""Causal multi-head attention block (qkv proj + attention + out proj) on 8
Trainium2 NeuronCores.

Sharding: core c = 2*b + hg handles batch b (of 4) and head-group hg (8 of 16
heads).  Each core computes qkv for its heads, causal attention, and a partial
out-projection (its 512 rows of w_out); the host sums the two head-group
partials per batch.

v1 layout (bf16 operands, fp32 PSUM):
  - x arrives pre-transposed from the host as xT [DIM, T] bf16; per t-quarter
    one DMA loads the 8 [128, 512] contraction chunks (no PE transposes).
  - weights arrive bf16, each as ONE large DMA (split across all 16 SDMA
    engines by the runtime).
  - Q^T/K^T come out of the projection as head-pair tiles [128 = 2 heads x 64,
    t]; V in natural [t, c] layout augmented with a ones column per head
    (V_aug), so P @ V_aug accumulates the numerator and softmax denominator
    together (no max-subtraction: scores ~ N(0,1), exp safe).
  - scores are computed transposed, S^T[k, q], two heads concurrently via PE
    row tiling (K=64 each) into one two-bank PSUM tile; exp (scale fused) is
    one ACT op per pair, narrowed on diagonal blocks; causal masking is a 0/1
    multiply on one 128-wide strip.
  - normalization per pair: the denominator row is cast to SBUF, partition-
    broadcast by two K=1 PE matmuls against a 0/1 selector row (no DMA),
    inverted with reciprocal_approx_fast at 128 lanes, and multiplied into
    the attention tile; the broadcast matmul's emission is DEFERRED into the
    next pair's stream so it never blocks the in-order PE queue.
  - PV matmuls run with lag-2 behind the scores and are split per head AROUND
    the next score pair, so every large LDWEIGHTS hides inside a long matmul
    (the ~4 ns row-group follower offers no LDW window).
  - emission is phase-interleaved: qkv quarter q+1 is pumped into attention
    block q; out_proj is deferred into the exp-bound tail phases (op(0) under
    att(2), op(1)+op(2) under att(3)) to keep the PE fed; the final out_proj
    block is split m-wise (heads 0-5 accumulated into SBUF early, heads 6-7
    joined after the last normalization) to shorten the tail.
"""

import sys

if "/opt/trn_rl_repo" not in sys.path:
    sys.path.insert(0, "/opt/trn_rl_repo")

import numpy as np

import concourse.bass as bass
import concourse.mybir as mybir
import concourse.tile as tile
from concourse import bacc
from concourse.bass_utils import run_bass_kernel_spmd

DIM = 1024
N_HEAD = 16
HD = 64
B, T = 4, 2048
HG = 8          # heads per core
CQ = HG * HD    # 512 feature columns per group
NCORES = 8
NT = T // 128   # 16 t-subtiles
NQ = T // 512   # 4 quarters / q-blocks

f32 = mybir.dt.float32
f32r = mybir.dt.float32r
bf16 = mybir.dt.bfloat16
Exp = mybir.ActivationFunctionType.Exp
AluAdd = mybir.AluOpType.add


def build_nc():
    nc = bacc.Bacc(None, target_bir_lowering=False)
    xt_d = nc.declare_dram_parameter("xt", [DIM, T], bf16, isOutput=False)
    wqk_d = nc.declare_dram_parameter("wqk", [DIM, 2 * CQ], bf16, isOutput=False)
    wv_d = nc.declare_dram_parameter("wv", [DIM, CQ], bf16, isOutput=False)
    wo_d = nc.declare_dram_parameter("wo", [CQ, DIM], bf16, isOutput=False)
    mv_d = nc.declare_dram_parameter("maskv", [128, NT], f32, isOutput=False)
    out_d = nc.declare_dram_parameter("out", [T, DIM], bf16, isOutput=True)

    with tile.TileContext(nc) as tc:
        with tc.tile_pool(name="pp", bufs=1) as pp, \
             tc.tile_pool(name="xq_p", bufs=2) as xq_p, \
             tc.tile_pool(name="qtp", bufs=2) as qtp, \
             tc.tile_pool(name="p_p", bufs=6) as p_p, \
             tc.tile_pool(name="at_p", bufs=3) as at_p, \
             tc.tile_pool(name="den_p", bufs=2) as den_p, \
             tc.tile_pool(name="bcs_p", bufs=2) as bcs_p, \
             tc.tile_pool(name="acc_p", bufs=1) as acc_p, \
             tc.tile_pool(name="out_p", bufs=2) as out_p, \
             tc.tile_pool(name="ps_aux", bufs=2, space="PSUM") as ps_aux, \
             tc.tile_pool(name="ps_s", bufs=2, space="PSUM") as ps_s, \
             tc.tile_pool(name="ps_pv", bufs=1, space="PSUM") as ps_pv:

            # ---- constants ----
            mv_sb = pp.tile([128, NT], f32, name="maskv_sb", tag="maskv_sb")
            nc.scalar.dma_start(out=mv_sb, in_=mv_d[:, :])
            # one 128x128 causal strip: keep where q_local >= k_local
            dstrip = pp.tile([128, 128], f32, name="dstrip", tag="dstrip")
            nc.gpsimd.memset(dstrip, 1.0)
            nc.gpsimd.affine_select(
                out=dstrip, in_=dstrip, compare_op=mybir.AluOpType.is_ge,
                fill=0.0, base=0, pattern=[[1, 128]], channel_multiplier=-1)
            onescol = pp.tile([128, HG], f32, name="onescol", tag="onescol")
            nc.vector.memset(onescol, 1.0)
            # head-half selector rows for the PE denominator broadcast:
            # bmh[0, 0:128] selects partitions 0-63, bmh[0, 128:256] selects
            # partitions 64-127 (K=1 matmuls: out[p, q] = bmh[p] * den[q])
            bmh = pp.tile([1, 256], bf16, name="bmh", tag="bmh")
            nc.vector.memset(bmh, 0.0)
            nc.vector.memset(bmh[0:1, 0:64], 1.0)
            nc.vector.memset(bmh[0:1, 192:256], 1.0)

            # ---- persistent weights: one big DMA each ----
            wqk_sb = pp.tile([128, 8, 2 * CQ], bf16, name="wqk_sb", tag="wqk_sb")
            wv_sb = pp.tile([128, 8, CQ], bf16, name="wv_sb", tag="wv_sb")
            wo_sb = pp.tile([128, 4, DIM], bf16, name="wo_sb", tag="wo_sb")
            # x quarter tiles [128, 8 kb chunks, 512]; first quarter loads now
            xts_cur = {}

            def load_x_quarter(q):
                xq = xq_p.tile([128, 8, 512], bf16, name=f"xq{q}", tag="xq")
                x3 = xt_d[:, q * 512:(q + 1) * 512].rearrange(
                    "(kb p) t -> p kb t", p=128)
                nc.sync.dma_start(out=xq[:, 0:4, :], in_=x3[:, 0:4, :])
                nc.scalar.dma_start(out=xq[:, 4:8, :], in_=x3[:, 4:8, :])
                xts_cur[q] = xq

            wqk3 = wqk_d.rearrange("(kb p) n -> p kb n", p=128)
            x3 = xt_d[:, 0:512].rearrange("(kb p) t -> p kb t", p=128)
            xq0 = xq_p.tile([128, 8, 512], bf16, name="xq0", tag="xq")
            xts_cur[0] = xq0
            # interleave 2-chunk pieces across both HWDGE rings so the first
            # qk unit's operands land as early as possible
            for i in range(4):
                eng = nc.sync if i % 2 == 0 else nc.scalar
                eng.dma_start(out=wqk_sb[:, 2 * i:2 * i + 2, :],
                              in_=wqk3[:, 2 * i:2 * i + 2, :])
                eng2 = nc.scalar if i % 2 == 0 else nc.sync
                eng2.dma_start(out=xq0[:, 2 * i:2 * i + 2, :],
                               in_=x3[:, 2 * i:2 * i + 2, :])
            nc.sync.dma_start(
                out=wv_sb, in_=wv_d.rearrange("(kb p) n -> p kb n", p=128))
            nc.sync.dma_start(
                out=wo_sb, in_=wo_d.rearrange("(m p) n -> p m n", p=128))

            # ---- persistent tensors ----
            kt = [pp.tile([128, T], bf16, name=f"kt{m}", tag=f"kt{m}") for m in range(4)]
            vaug = [pp.tile([128, HG * 65], bf16, name=f"vaug{t}", tag=f"vaug{t}")
                    for t in range(NT)]


            qt_cur = {}    # quarter -> [4 pair tiles [128, 512]]
            ats_cur = {}   # qb -> [4 pair tiles [128, 512]]

            # ---------- qkv quarter units ----------
            def qkv_units(q, with_xload=None, qk_too=True):
                units = []
                if with_xload is not None:
                    units.append(lambda: load_x_quarter(with_xload))
                if qk_too:
                    qt_cur[q] = [None] * 4

                def qk_evac(m, pq):
                    if m < 4:
                        qtile = qtp.tile([128, 512], bf16, name=f"qt{m}", tag=f"qt{m}")
                        nc.vector.tensor_copy(qtile, pq)
                        qt_cur[q][m] = qtile
                    else:
                        nc.vector.tensor_copy(
                            kt[m - 4][:, q * 512:(q + 1) * 512], pq)

                # each unit is HALF a contraction (~850ns of PE): fine filler
                # granularity keeps the exp stream fed (the score-psum depth
                # only buffers ~2 k-tiles of ACT-ahead, so >2us filler bursts
                # starve the ACT engine).  Halves stay adjacent in the filler
                # list, so the open PSUM group never deadlocks.
                qk_open = {}

                def qk_half(m, half):
                    xq = xts_cur[q]
                    if half == 0:
                        qk_open[m] = ps_aux.tile(
                            [128, 512], f32, name="mm", tag="aux")
                    pq = qk_open[m]
                    for kb in range(4 * half, 4 * half + 4):
                        nc.tensor.matmul(
                            pq, wqk_sb[:, kb, m * 128:(m + 1) * 128],
                            xq[:, kb, :], start=(kb == 0), stop=(kb == 7))
                    if half == 1:
                        qk_evac(m, pq)
                if qk_too:
                    for m in range(8):
                        units.append(lambda m=m: qk_half(m, 0))
                        units.append(lambda m=m: qk_half(m, 1))

                def v_evac(ti, pv):
                    vt = vaug[q * 4 + ti]
                    vt3 = vt.rearrange("p (h w) -> p h w", w=65)
                    nc.vector.tensor_copy(
                        vt3[:, :, 0:64], pv.rearrange("p (h w) -> p h w", w=64))
                    nc.vector.tensor_copy(
                        vt3[:, :, 64:65], onescol.rearrange("p (h w) -> p h w", w=1))
                    nc.vector.tensor_scalar_mul(
                        vt, vt, mv_sb[:, (q * 4 + ti):(q * 4 + ti + 1)])

                v_open = {}

                def v_half(ti, half):
                    xq = xts_cur[q]
                    if half == 0:
                        v_open[ti] = ps_aux.tile(
                            [128, 512], f32, name="mm", tag="aux")
                    pv = v_open[ti]
                    for kb in range(4 * half, 4 * half + 4):
                        nc.tensor.matmul(
                            pv, xq[:, kb, ti * 128:(ti + 1) * 128],
                            wv_sb[:, kb, :], start=(kb == 0), stop=(kb == 7))
                    if half == 1:
                        v_evac(ti, pv)
                for ti in range(4):
                    units.append(lambda ti=ti: v_half(ti, 0))
                    units.append(lambda ti=ti: v_half(ti, 1))
                return units

            # ---------- out_proj units ----------
            ob_cur = {}

            def op_mm(po, qb, m, ti, nb, start, stop):
                nc.tensor.matmul(
                    po, ats_cur[qb][m][:, ti * 128:(ti + 1) * 128],
                    wo_sb[:, m, nb * 512:(nb + 1) * 512],
                    start=start, stop=stop)

            def op_store(qb, ti, ob):
                t0 = (qb * 4 + ti) * 128
                nc.sync.dma_start(out=out_d[t0:t0 + 128, :], in_=ob)

            def outproj_units(qb):
                units = []

                def op_unit(ti, nb):
                    po = ps_aux.tile([128, 512], f32, name="mm", tag="aux")
                    for m in range(4):
                        op_mm(po, qb, m, ti, nb, start=(m == 0), stop=(m == 3))
                    if nb == 0:
                        ob = out_p.tile([128, DIM], bf16, name="ob", tag="ob")
                        ob_cur[qb, ti] = ob
                    ob = ob_cur[qb, ti]
                    nc.vector.tensor_copy(ob[:, nb * 512:(nb + 1) * 512], po)
                    if nb == 1:
                        op_store(qb, ti, ob)
                for ti in range(4):
                    for nb in range(2):
                        units.append(lambda ti=ti, nb=nb: op_unit(ti, nb))
                return units

            # last q-block: heads 0-5 (m=0..2) accumulated into SBUF before the
            # final pair normalizes; m=3 joins after.
            acc_sb = {}

            def op_partial_units(qb):
                units = []

                def part_unit(ti, nb):
                    po = ps_aux.tile([128, 512], f32, name="mm", tag="aux")
                    for m in range(3):
                        op_mm(po, qb, m, ti, nb, start=(m == 0), stop=(m == 2))
                    acc = acc_p.tile([128, 512], f32, name=f"acc{ti}{nb}",
                                     tag=f"acc{ti}{nb}")
                    nc.vector.tensor_copy(acc, po)
                    acc_sb[ti, nb] = acc
                for ti in range(4):
                    for nb in range(2):
                        units.append(lambda ti=ti, nb=nb: part_unit(ti, nb))
                return units

            def op_final_units(qb):
                units = []

                def fin_unit(ti, nb):
                    po = ps_aux.tile([128, 512], f32, name="mm", tag="aux")
                    op_mm(po, qb, 3, ti, nb, start=True, stop=True)
                    if nb == 0:
                        ob = out_p.tile([128, DIM], bf16, name="ob", tag="ob")
                        ob_cur[qb, ti] = ob
                    ob = ob_cur[qb, ti]
                    nc.vector.tensor_tensor(
                        ob[:, nb * 512:(nb + 1) * 512], po, acc_sb[ti, nb], AluAdd)
                    if nb == 1:
                        op_store(qb, ti, ob)
                for ti in range(4):
                    for nb in range(2):
                        units.append(lambda ti=ti, nb=nb: fin_unit(ti, nb))
                return units

            # ---------- attention pair ----------
            # finishing (denominator broadcast + normalize) is deferred so
            # the tiny PE matmul never blocks the next pair's score matmuls
            # in the in-order PE queue
            deferred = []

            def flush_deferred():
                while deferred:
                    deferred.pop(0)()

            def att_pair(qb, m, pump):
                nk = 4 * (qb + 1)
                pvp = ps_pv.tile([65, 1024], f32, name="pv", tag="pv")

                def pv_h(pk, pt, w0, h, stop):
                    # masked q-columns [0:w0) of this k-tile are exactly zero:
                    # skip them; PSUM accumulation keeps their prior value
                    nc.tensor.matmul(
                        pvp[:, h * 512 + w0:h * 512 + 512],
                        vaug[pk][:, (2 * m + h) * 65:(2 * m + h + 1) * 65],
                        pt[:, h * 512 + w0:h * 512 + 512],
                        start=(pk == 0), stop=stop)

                pending = []
                for kti in range(nk):
                    j = kti - 4 * qb
                    w0 = 128 * j if j > 0 else 0
                    # interleave the lagged PV pair AROUND the score pair so
                    # every big LDWEIGHTS hides inside a long matmul (the 4 ns
                    # row-group follower offers no LDW window)
                    if len(pending) > 2:
                        pv_h(*pending[0], h=0, stop=False)
                    sp = ps_s.tile([128, 1024], f32, name="s", tag="s")
                    nc.tensor.matmul(
                        sp[:, w0:512],
                        kt[m][0:64, kti * 128:(kti + 1) * 128],
                        qt_cur[qb][m][0:64, w0:512], start=True, stop=True)
                    nc.tensor.matmul(
                        sp[:, 512 + w0:1024],
                        kt[m][64:128, kti * 128:(kti + 1) * 128],
                        qt_cur[qb][m][64:128, w0:512], start=True, stop=True)
                    pump()
                    if len(pending) > 2:
                        pv_h(*pending.pop(0), h=1, stop=False)
                    if kti == 1:
                        flush_deferred()
                    ppt = p_p.tile([128, 1024], bf16, name="p", tag="p")
                    p3 = ppt.rearrange("p (h w) -> p h w", w=512)
                    s3 = sp.rearrange("p (h w) -> p h w", w=512)
                    if j < 0:
                        nc.scalar.activation(p3, s3, Exp, scale=0.125)
                    else:
                        nc.scalar.activation(
                            p3[:, :, w0:512], s3[:, :, w0:512], Exp, scale=0.125)
                        for h in range(2):
                            nc.vector.tensor_mul(
                                ppt[:, h * 512 + w0:h * 512 + w0 + 128],
                                ppt[:, h * 512 + w0:h * 512 + w0 + 128],
                                dstrip)
                    pending.append((kti, ppt, w0))
                    pump()
                while pending:
                    last = len(pending) == 1
                    pv_h(*pending[0], h=0, stop=False)
                    pv_h(*pending.pop(0), h=1, stop=last)

                atm = at_p.tile([128, 512], bf16, name=f"at{m}", tag=f"at{m}")
                ats_cur[qb][m] = atm
                last = (qb == NQ - 1 and m == 3)
                # denominator row first so the deferred broadcast matmul's
                # operand lands as early as possible; for the last pair split
                # it across scalar+DVE so the h0 broadcast starts after only
                # half the copy, with the atm copies also on the DVE queue
                denb = den_p.tile([1, 1024], bf16, name="denb", tag="denb")
                if last:
                    nc.scalar.copy(denb[0:1, 0:512], pvp[64:65, 0:512])
                    nc.vector.tensor_copy(denb[0:1, 512:1024],
                                          pvp[64:65, 512:1024])
                else:
                    nc.vector.tensor_copy(denb, pvp[64:65, :])
                nc.vector.tensor_copy(atm[0:64, :], pvp[0:64, 0:512])
                nc.vector.tensor_copy(atm[64:128, :], pvp[0:64, 512:1024])

                def fin():
                    # partition-broadcast via two K=1 PE matmuls
                    # (bmh half ^T @ den half), then reciprocal at 128 lanes
                    bfull = ps_s.tile([128, 1024], f32, name="s", tag="s")
                    for h in range(2):
                        nc.tensor.matmul(
                            bfull[:, 0:512], bmh[0:1, h * 128:(h + 1) * 128],
                            denb[0:1, h * 512:(h + 1) * 512],
                            start=(h == 0), stop=(h == 1))
                    bcs = bcs_p.tile([128, 512], f32, name="bcs", tag="bcs")
                    nc.vector.reciprocal_approx_fast(out=bcs, in_=bfull[:, 0:512])
                    if last:
                        # chunk the normalize so each final out_proj matmul
                        # starts as soon as its ti-columns are scaled
                        for c in range(4):
                            nc.vector.tensor_mul(
                                atm[:, c * 128:(c + 1) * 128],
                                atm[:, c * 128:(c + 1) * 128],
                                bcs[:, c * 128:(c + 1) * 128])
                    else:
                        nc.vector.tensor_mul(atm, atm, bcs)
                deferred.append(fin)

            def run_phase(tasks, fillers, n_units):
                """tasks: closures taking pump(); fillers pumped proportionally."""
                fillers = [flush_deferred] + fillers
                nf = len(fillers)
                state = {"fi": 0, "ai": 0}

                def pump():
                    state["ai"] += 1
                    while state["fi"] * n_units < state["ai"] * nf \
                            and state["fi"] < nf:
                        fillers[state["fi"]]()
                        state["fi"] += 1
                for t in tasks:
                    t(pump)
                while state["fi"] < nf:
                    fillers[state["fi"]]()
                    state["fi"] += 1

            # ---------------- emission schedule ----------------
            # quarter 0 runs during the initial DMA: kb4-7 chunks arrive
            # ~4us after kb0-3, so split each qk contraction into a kb0-3
            # group (evacuated as a bf16 partial) and a kb4-7 group merged
            # in by a DVE add -- the PE stays continuously busy from the
            # first chunk and holds its p-state ramp
            qt_cur[0] = [None] * 4

            def qk0_dest(m):
                return (qt_cur[0][m] if m < 4
                        else kt[m - 4][:, 0:512])

            def qk0_ab(m, half):
                xq = xts_cur[0]
                pq = ps_aux.tile([128, 512], f32, name="mm", tag="aux")
                for kb in range(4 * half, 4 * half + 4):
                    nc.tensor.matmul(
                        pq, wqk_sb[:, kb, m * 128:(m + 1) * 128],
                        xq[:, kb, :], start=(kb == 4 * half),
                        stop=(kb == 4 * half + 3))
                if half == 0:
                    if m < 4:
                        qt_cur[0][m] = qtp.tile(
                            [128, 512], bf16, name=f"qt{m}", tag=f"qt{m}")
                    nc.vector.tensor_copy(qk0_dest(m), pq)
                else:
                    d = qk0_dest(m)
                    nc.vector.tensor_tensor(d, pq, d, AluAdd)

            for m in range(8):
                qk0_ab(m, 0)
            for m in range(8):
                qk0_ab(m, 1)
            for u in qkv_units(0, qk_too=False):
                u()
            for qb in range(NQ):
                ats_cur[qb] = [None] * 4

            def phase_tasks(qb, ms):
                def mk(m):
                    def t(pump):
                        att_pair(qb, m, pump)
                    return t
                return [mk(m) for m in ms]

            # phase 0: att(0) + qkv(1)
            run_phase(phase_tasks(0, range(4)),
                      qkv_units(1, with_xload=1), 32)
            # phase 1: att(1) + qkv(2)
            run_phase(phase_tasks(1, range(4)),
                      qkv_units(2, with_xload=2), 64)
            # phase 2: att(2) + op(0) + qkv(3)
            run_phase(phase_tasks(2, range(4)),
                      outproj_units(0) + qkv_units(3, with_xload=3), 96)
            # phase 3: att(3) pairs 0-2 + op(1) + op(2)  (out_proj deferred
            # into the exp-bound tail phases to keep the PE fed)
            run_phase(phase_tasks(3, range(3)),
                      outproj_units(1) + outproj_units(2), 96)
            # phase 4: att(3) pair 3 + partial out_proj over pairs 0-2
            run_phase(phase_tasks(3, [3]),
                      op_partial_units(3), 32)
            # tail: only the m=3 out_proj slices depend on the last pair
            flush_deferred()
            for u in op_final_units(3):
                u()
    nc.finalize()
    return nc


_NC_CACHE = {}


def _get_nc():
    if "nc" not in _NC_CACHE:
        _NC_CACHE["nc"] = build_nc()
    return _NC_CACHE["nc"]


def _make_in_maps(x, w_qkv, w_out, attn_mask):
    np_bf16 = mybir.dt.np(bf16)
    x = np.asarray(x, dtype=np.float32)
    w_qkv = np.asarray(w_qkv, dtype=np.float32)
    w_out = np.asarray(w_out, dtype=np.float32)
    am = np.asarray(attn_mask)
    in_maps = []
    for c in range(NCORES):
        b, hg = c // 2, c % 2
        wqk_c = np.ascontiguousarray(np.concatenate(
            [w_qkv[:, hg * CQ:(hg + 1) * CQ],
             w_qkv[:, DIM + hg * CQ:DIM + (hg + 1) * CQ]], axis=1)).astype(np_bf16)
        wv_c = np.ascontiguousarray(
            w_qkv[:, 2 * DIM + hg * CQ:2 * DIM + (hg + 1) * CQ]).astype(np_bf16)
        wo_c = np.ascontiguousarray(w_out[hg * CQ:(hg + 1) * CQ, :]).astype(np_bf16)
        mv_c = np.ascontiguousarray(
            am[b].astype(np.float32).reshape(NT, 128).T)
        xt_c = np.ascontiguousarray(x[b].T).astype(np_bf16)
        in_maps.append({
            "xt": xt_c,
            "wqk": wqk_c,
            "wv": wv_c,
            "wo": wo_c,
            "maskv": mv_c,
        })
    return in_maps


def run(x, w_qkv, w_out, attn_mask, trace=False):
    nc = _get_nc()
    in_maps = _make_in_maps(x, w_qkv, w_out, attn_mask)
    res = run_bass_kernel_spmd(nc, in_maps, list(range(NCORES)), trace=trace)
    outs = [np.asarray(res.results[c]["out"], dtype=np.float32)
            for c in range(NCORES)]
    full = np.stack([outs[2 * b] + outs[2 * b + 1] for b in range(B)], axis=0)
    return full.astype(np.float32), res


def kernel(x, w_qkv, w_out, attn_mask):
    full, _ = run(x, w_qkv, w_out, attn_mask, trace=False)
    return full
